# revision 10
# baseline (speedup 1.0000x reference)
"""GNN message-passing kernel for 8 Trainium2 NeuronCores.

Computes: relu(concat([x @ Wx + bx, segment_sum(edge_attr, src) @ We + be], axis=1))

Strategy (graph-parallel, per the sharding hint):
  - Nodes are sharded 8 ways (12500 per core); edges are bucketed by the
    core that owns their source node, so the segment-sum is core-local.
  - On-device segment-sum: edges are host-bucketed into 64-node windows
    (padded to K chunks of 128 edges each). Four windows form a quad: their
    aggT strips live in the four 32-partition strips of one PSUM tile, and
    the four one-hot matmuls per chunk step run concurrently via PE
    col-tiling (stationary = edge_attr chunk [128,32] bf16 in col-group j,
    moving = one-hot [128,64] bf16). One-hots for a whole quad are built in
    a single DVE is_equal over [128, 4K*64].
  - Epilogue per 128 nodes: two row+col-tiled he matmuls (K=32) read the
    aggT strips directly, a K=1 outer-product matmul adds both biases, and
    the x @ Wx matmul accumulates into the same PSUM tile ([Wx | 0] /
    [0 | We] column layouts fold the concat); ACT applies relu from PSUM.
"""

import sys

sys.path.insert(0, "/opt/trn_rl_repo")

import numpy as np
import ml_dtypes

from concourse import mybir, bacc
import concourse.tile as tile
from concourse.bass_utils import run_bass_kernel_spmd

# Problem constants (hardcoded per the nn_NodeCentric spec)
N = 100_000
E = 1_600_000
FE = 32
FX = 128
OX = 128
OE = 128
NCORES = 8
NPC = N // NCORES          # nodes per core = 12500
W = 64                     # node window (one-hot moving width)
SUP = 512                  # supertile = 8 windows = 2 quads
NSUP = (NPC + SUP - 1) // SUP          # 25
NPC_PAD = NSUP * SUP                   # 12800
WIN_PER_CORE = NPC_PAD // W            # 200
WPS = SUP // W                         # 8 windows per supertile

BF16 = ml_dtypes.bfloat16

_program_cache: dict[int, object] = {}
last_results = None  # BassKernelResults of the most recent run (for test harness)


def _build_program(K: int):
    """Build the (identical-across-cores) Bass program for K chunks/window."""
    C = WIN_PER_CORE * K        # chunks per core
    CPS = WPS * K               # chunks per supertile
    QK = 4 * K                  # chunks per quad

    nc = bacc.Bacc("TRN2", target_bir_lowering=False, debug=False)
    f32 = mybir.dt.float32
    bf16 = mybir.dt.bfloat16
    Relu = mybir.ActivationFunctionType.Relu
    Copy = mybir.ActivationFunctionType.Copy

    d_attr = nc.dram_tensor("attr", [128, C * 32], bf16, kind="ExternalInput")
    d_idx = nc.dram_tensor("idx", [128, C], bf16, kind="ExternalInput")
    d_xt = nc.dram_tensor("xt", [128, NPC_PAD], bf16, kind="ExternalInput")
    d_wx0 = nc.dram_tensor("wx0", [128, 256], bf16, kind="ExternalInput")
    d_webb4 = nc.dram_tensor("webb4", [128, 256], bf16, kind="ExternalInput")
    d_bias = nc.dram_tensor("bias", [1, 256], bf16, kind="ExternalInput")
    d_iota = nc.dram_tensor("iota", [128, W], bf16, kind="ExternalInput")
    d_out = nc.dram_tensor("out", [NPC_PAD, 256], f32, kind="ExternalOutput")

    with tile.TileContext(nc) as tc:
        with (
            tc.tile_pool(name="const", bufs=1) as constp,
            tc.tile_pool(name="attr", bufs=3) as attrp,
            tc.tile_pool(name="oh", bufs=3) as ohp,
            tc.tile_pool(name="agg", bufs=4) as aggp,
            tc.tile_pool(name="outs", bufs=3) as outp,
            tc.tile_pool(name="psagg", bufs=4, space="PSUM") as psaggp,
            tc.tile_pool(name="psout", bufs=2, space="PSUM") as psoutp,
        ):
            t_idx = constp.tile([128, C], bf16)
            nc.sync.dma_start(out=t_idx[:], in_=d_idx[:])
            t_xt = constp.tile([128, NPC_PAD], bf16)
            nc.sync.dma_start(out=t_xt[:], in_=d_xt[:])
            t_wx0 = constp.tile([128, 256], bf16)
            nc.sync.dma_start(out=t_wx0[:], in_=d_wx0[:])
            t_webb4 = constp.tile([128, 256], bf16)
            nc.sync.dma_start(out=t_webb4[:], in_=d_webb4[:])
            t_bias = constp.tile([1, 256], bf16)
            nc.sync.dma_start(out=t_bias[:], in_=d_bias[:])
            t_iota = constp.tile([128, W], bf16)
            nc.sync.dma_start(out=t_iota[:], in_=d_iota[:])
            t_one = constp.tile([1, 128], bf16)
            nc.vector.memset(t_one[:], 1.0)

            for s in range(NSUP):
                t_attr = attrp.tile([128, CPS * 32], bf16)
                nc.sync.dma_start(
                    out=t_attr[:], in_=d_attr[:, s * CPS * 32 : (s + 1) * CPS * 32]
                )
                t_aggs = []
                for q in range(2):
                    cw = s * CPS + q * QK   # first chunk of this quad (global)
                    oh = ohp.tile([128, QK * W], bf16)
                    nc.vector.tensor_tensor(
                        out=oh[:].rearrange("p (c f) -> p c f", f=W),
                        in0=t_iota[:].unsqueeze(1).broadcast_to([128, QK, W]),
                        in1=t_idx[:, cw : cw + QK].to_broadcast([128, QK, W]),
                        op=mybir.AluOpType.is_equal,
                    )
                    ps_q = psaggp.tile([128, W], f32)
                    for k in range(K):
                        for j in range(4):
                            ci = (q * 4 + j) * K + k   # chunk within supertile
                            nc.tensor.matmul(
                                out=ps_q[32 * j : 32 * (j + 1), :],
                                lhsT=t_attr[:, ci * 32 : (ci + 1) * 32],
                                rhs=oh[:, (j * K + k) * W : (j * K + k + 1) * W],
                                start=(k == 0),
                                stop=(k == K - 1),
                                tile_position=(0, 32 * j),
                                skip_group_check=True,
                            )
                    t_agg = aggp.tile([128, W], bf16)
                    nc.scalar.activation(out=t_agg[:], in_=ps_q[:], func=Copy)
                    t_aggs.append(t_agg)

                t_out = outp.tile([128, 1024], f32)
                for g in range(4):          # 128-node groups (windows 2g, 2g+1)
                    t_agg = t_aggs[g // 2]
                    ps_o = psoutp.tile([128, 256], f32)
                    for h in range(2):      # two 64-node windows in the group
                        j = 2 * (g % 2) + h  # strip within the quad
                        nc.tensor.matmul(
                            out=ps_o[64 * h : 64 * (h + 1), :],
                            lhsT=t_agg[32 * j : 32 * (j + 1), :],
                            rhs=t_webb4[32 * j : 32 * (j + 1), :],
                            start=True,
                            stop=False,
                            tile_position=(32 * j, 64 * h),
                            skip_group_check=True,
                        )
                    nc.tensor.matmul(
                        out=ps_o[:],
                        lhsT=t_one[:],
                        rhs=t_bias[:],
                        start=False,
                        stop=False,
                        tile_position=(0, 0),
                        skip_group_check=True,
                    )
                    nc.tensor.matmul(
                        out=ps_o[:],
                        lhsT=t_xt[:, s * SUP + g * 128 : s * SUP + (g + 1) * 128],
                        rhs=t_wx0[:],
                        start=False,
                        stop=True,
                        skip_group_check=True,
                    )
                    nc.scalar.activation(
                        out=t_out[:, g * 256 : (g + 1) * 256], in_=ps_o[:], func=Relu
                    )
                nc.sync.dma_start(
                    out=d_out[s * SUP : (s + 1) * SUP, :].rearrange(
                        "(j p) f -> p j f", p=128
                    ),
                    in_=t_out[:].rearrange("p (j f) -> p j f", f=256),
                )
    nc.compile()
    return nc


def kernel(x, edge_index, edge_attr, Wx, bx, We, be):
    x = np.asarray(x, dtype=np.float32)
    edge_attr = np.asarray(edge_attr, dtype=np.float32)
    Wx = np.asarray(Wx, dtype=np.float32)
    bx = np.asarray(bx, dtype=np.float32)
    We = np.asarray(We, dtype=np.float32)
    be = np.asarray(be, dtype=np.float32)
    src = np.asarray(edge_index[0], dtype=np.int64)

    # ---- host-side edge bucketing (core -> 64-node window -> 128-edge chunks)
    core = src // NPC                       # 0..7
    rel = src - core * NPC                  # 0..12499
    win = rel // W                          # 0..195
    within = (rel - win * W).astype(np.float32)  # 0..63
    bucket = core * WIN_PER_CORE + win      # global (core, window) id
    nbuckets = NCORES * WIN_PER_CORE
    counts = np.bincount(bucket, minlength=nbuckets)
    K = max(2, int(-(-counts.max() // 128)))   # chunks per window (uniform)
    K += K & 1                                 # even K keeps chunk offsets aligned
    EPW = 128 * K                              # padded edges per window

    order = np.argsort(bucket, kind="stable")
    sbucket = bucket[order]
    starts = np.zeros(nbuckets + 1, dtype=np.int64)
    starts[1:] = np.cumsum(counts)
    pos = np.arange(E, dtype=np.int64) - starts[sbucket]
    dest = sbucket * EPW + pos

    idx_pad = np.full(nbuckets * EPW, W, dtype=np.float32)  # W==64 never matches iota
    idx_pad[dest] = within[order]
    attr_pad = np.zeros((nbuckets * EPW, FE), dtype=BF16)
    attr_pad[dest] = edge_attr[order].astype(BF16)

    C = WIN_PER_CORE * K
    # ---- per-core input maps
    wx0 = np.zeros((128, 256), dtype=BF16)
    wx0[:, 0:128] = Wx.astype(BF16)
    webb4 = np.zeros((128, 256), dtype=BF16)
    for j in range(4):
        webb4[32 * j : 32 * (j + 1), 128:256] = We.astype(BF16)
    bias = np.concatenate([bx, be]).reshape(1, 256).astype(BF16)
    iota = np.broadcast_to(np.arange(W, dtype=np.float32), (128, W)).astype(BF16).copy()

    in_maps = []
    for c in range(NCORES):
        a = attr_pad[c * C * 128 : (c + 1) * C * 128]          # [C*128, 32] bf16
        attr_slab = np.ascontiguousarray(
            a.reshape(C, 128, FE).transpose(1, 0, 2).reshape(128, C * FE)
        )
        idxT = np.ascontiguousarray(
            idx_pad[c * C * 128 : (c + 1) * C * 128].reshape(C, 128).T
        ).astype(BF16)
        xpad = np.zeros((NPC_PAD, FX), dtype=np.float32)
        xpad[:NPC] = x[c * NPC : (c + 1) * NPC]
        xT = np.ascontiguousarray(xpad.T.astype(BF16))          # [128, NPC_PAD]
        in_maps.append(
            {
                "attr": attr_slab,
                "idx": idxT,
                "xt": xT,
                "wx0": wx0,
                "webb4": webb4,
                "bias": bias,
                "iota": iota,
            }
        )

    if K not in _program_cache:
        _program_cache[K] = _build_program(K)
    nc = _program_cache[K]

    res = run_bass_kernel_spmd(nc, in_maps, core_ids=list(range(NCORES)))
    global last_results
    last_results = res
    out = np.concatenate([res.results[c]["out"][:NPC] for c in range(NCORES)], axis=0)
    return out
